# revision 1
# baseline (speedup 1.0000x reference)
"""Trainium2 Bass kernel for nn_MixquantLinear: O = ((dequant4(V) * S) @ dequant4(U)).T.

Output O is [4096, 4096] fp32, built purely from the GPTQ-quantized weights
(the activation input `x` is dead code in the reference and never touches the
device). Sharding: 4 slices over output rows (o) x 2 over output cols (i)
-> 8 cores, no collectives; host concatenates the blocks.

Per core:
  - unpack 4-bit nibbles (vector engine shift+mask on int32 words)
  - dequant as q*a + b with a = scale (S folded into V's), b = -(z+1)*scale;
    the b tables are built in the zeros' natural packed layout and
    PE-transposed once (tiny), so no wide dequant table work exists
  - affine chunks split between vector and scalar engines
  - U slice PE-transposed into [rank, out] lhsT: 4 transposes per PSUM tile
    at bank-aligned offsets + one strided copy
  - fp16 matmuls (k-tiles of 128, N=512) accumulating fp32 in PSUM;
    first wave k-layered so it can chase the strip-0 dequant
Host-side work is layout-only (slicing/transposing packed int32 words and
fp32 scale tables, concatenating outputs).
"""

import numpy as np

import concourse.bass as bass
import concourse.mybir as mybir
import concourse.tile as tile
from concourse import bacc
from concourse.bass_utils import run_bass_kernel_spmd
from concourse.masks import make_identity

IN_SIZE = 4096
OUT_SIZE = 4096
RANK = 1024
GROUPSIZE = 128
PACK = 8
P_O = 4
P_I = 2
O_SL = OUT_SIZE // P_O    # 1024
I_SL = IN_SIZE // P_I     # 2048
N_CORES = P_O * P_I
KT = RANK // 128          # 8
RT = KT
OT = O_SL // 128          # 8
N_STRIPS = 2
STRIP = I_SL // N_STRIPS  # 1024
GV = I_SL // GROUPSIZE    # 16
GU = RANK // GROUPSIZE    # 8

F16 = mybir.dt.float16
F32 = mybir.dt.float32
I32 = mybir.dt.int32
Alu = mybir.AluOpType
Act = mybir.ActivationFunctionType

_NC_CACHE = None
TRACE = False
LAST_RESULTS = None

V_DVE, V_ACT = 2, 3   # V-affine chunk split DVE:ACT
U_DVE, U_ACT = 1, 1   # U-affine chunk split


def _build_nc():
    nc = bacc.Bacc("TRN2", target_bir_lowering=False)

    qvt = nc.dram_tensor("qvt", [128, RT * (I_SL // PACK)], I32, kind="ExternalInput")
    svt = nc.dram_tensor("svt", [128, RT * GV], F32, kind="ExternalInput")
    qzv = nc.dram_tensor("qzv", [GV, RANK // PACK], I32, kind="ExternalInput")
    sv_g = nc.dram_tensor("sv_g", [GV, RANK], F32, kind="ExternalInput")
    qut = nc.dram_tensor("qut", [128, OT * (RANK // PACK)], I32, kind="ExternalInput")
    sut = nc.dram_tensor("sut", [128, OT * GU], F32, kind="ExternalInput")
    qzu = nc.dram_tensor("qzu", [GU, O_SL // PACK], I32, kind="ExternalInput")
    su_g = nc.dram_tensor("su_g", [GU, O_SL], F32, kind="ExternalInput")
    s_in = nc.dram_tensor("s", [128, RT], F32, kind="ExternalInput")
    out = nc.dram_tensor("out", [O_SL, I_SL], F32, kind="ExternalOutput")

    cnt = {"v": 0, "u": 0, "cp": 0}

    def affine(phase, out_ap, in_ap, a_col, b_col):
        """out = in * a + b; weighted split between DVE and ACT."""
        dve_w, act_w = (V_DVE, V_ACT) if phase == "v" else (U_DVE, U_ACT)
        i = cnt[phase]
        cnt[phase] += 1
        if i % (dve_w + act_w) < dve_w:
            nc.vector.tensor_scalar(
                out=out_ap, in0=in_ap, scalar1=a_col, scalar2=b_col,
                op0=Alu.mult, op1=Alu.add)
        else:
            nc.scalar.activation(out_ap, in_ap, Act.Identity, bias=b_col, scale=a_col)

    def copy_alt(out_ap, in_ap):
        cnt["cp"] += 1
        if cnt["cp"] % 2 == 0:
            nc.scalar.copy(out_ap, in_ap)
        else:
            nc.vector.tensor_copy(out_ap, in_ap)

    with tile.TileContext(nc) as tc:
        with (
            tc.tile_pool(name="const", bufs=1) as cp,
            tc.tile_pool(name="nibs", bufs=4) as nibp,
            tc.tile_pool(name="outsb", bufs=8) as outp,
        ):
            qvt_sb = cp.tile([128, RT * (I_SL // PACK)], I32, tag="qvt")
            qut_sb = cp.tile([128, OT * (RANK // PACK)], I32, tag="qut")
            svt_sb = cp.tile([128, RT * GV], F32, tag="svt")
            sut_sb = cp.tile([128, OT * GU], F32, tag="sut")
            sv_g_sb = cp.tile([GV, RANK], F32, tag="svg")
            su_g_sb = cp.tile([GU, O_SL], F32, tag="sug")
            s_sb = cp.tile([128, RT], F32, tag="s")
            qzv_sb = cp.tile([GV, RANK // PACK], I32, tag="qzv")
            qzu_sb = cp.tile([GU, O_SL // PACK], I32, tag="qzu")
            zv_unp = cp.tile([GV, RANK], I32, tag="zvu")
            zu_unp = cp.tile([GU, O_SL], I32, tag="zuu")
            bg_v = cp.tile([GV, RANK], F32, tag="bgv")
            bg_u = cp.tile([GU, O_SL], F32, tag="bgu")
            av = cp.tile([128, RT * GV], F32, tag="av")
            bvnv = cp.tile([128, RT * GV], F32, tag="bvnv")
            bvnu = cp.tile([128, OT * GU], F32, tag="bvnu")
            id16 = cp.tile([128, 128], F16, tag="id16")
            id32 = cp.tile([128, 128], F32, tag="id32")
            wut = cp.tile([128, OT * RANK], F16, tag="wut")
            lhsT = cp.tile([128, KT * O_SL], F16, tag="lhsT")
            rhs = [cp.tile([128, RT * STRIP], F16, tag=f"rhs{s}", name=f"rhs{s}")
                   for s in range(N_STRIPS)]

            make_identity(nc, id16[:])
            make_identity(nc, id32[:])
            # DMA order: zero tables first (gate the bvn chain), then qut
            # (first big DVE consumer), then qvt per r-tile, then scales.
            nc.sync.dma_start(out=qzv_sb[:], in_=qzv[:])
            nc.sync.dma_start(out=qzu_sb[:], in_=qzu[:])
            nc.sync.dma_start(out=sv_g_sb[:], in_=sv_g[:])
            nc.sync.dma_start(out=su_g_sb[:], in_=su_g[:])
            nc.sync.dma_start(out=s_sb[:], in_=s_in[:])
            for h in range(4):
                w = OT * (RANK // PACK) // 4
                nc.sync.dma_start(out=qut_sb[:, h * w:(h + 1) * w],
                                  in_=qut[:, h * w:(h + 1) * w])
            nc.sync.dma_start(out=svt_sb[:], in_=svt[:])
            nc.sync.dma_start(out=sut_sb[:], in_=sut[:])
            for rt in range(RT):
                w = I_SL // PACK
                nc.sync.dma_start(out=qvt_sb[:, rt * w:(rt + 1) * w],
                                  in_=qvt[:, rt * w:(rt + 1) * w])

            # ---- zeros -> b tables: -(z+1)*scale in natural [g, x] layout ----
            zvu_r = zv_unp[:].rearrange("p (w j) -> p w j", j=PACK)
            zuu_r = zu_unp[:].rearrange("p (w j) -> p w j", j=PACK)
            for j in range(PACK):
                nc.vector.tensor_scalar(
                    out=zuu_r[:, :, j], in0=qzu_sb[:], scalar1=4 * j, scalar2=15,
                    op0=Alu.logical_shift_right, op1=Alu.bitwise_and)
            for j in range(PACK):
                nc.vector.tensor_scalar(
                    out=zvu_r[:, :, j], in0=qzv_sb[:], scalar1=4 * j, scalar2=15,
                    op0=Alu.logical_shift_right, op1=Alu.bitwise_and)
            # bg = -(z+1) then *= scale_g  (fp ops; int32 input auto-converts)
            nc.vector.tensor_scalar(out=bg_u[:], in0=zu_unp[:], scalar1=1.0,
                                    scalar2=-1.0, op0=Alu.add, op1=Alu.mult)
            nc.vector.tensor_tensor(bg_u[:], bg_u[:], su_g_sb[:], Alu.mult)
            nc.vector.tensor_scalar(out=bg_v[:], in0=zv_unp[:], scalar1=1.0,
                                    scalar2=-1.0, op0=Alu.add, op1=Alu.mult)
            nc.vector.tensor_tensor(bg_v[:], bg_v[:], sv_g_sb[:], Alu.mult)

            with tc.tile_pool(name="zps", bufs=2, space="PSUM") as zps:
                for t in range(OT):  # U first: it gates the lhsT chain
                    pt = zps.tile([128, GU], F32, tag="zp", name="zp")
                    nc.tensor.transpose(pt[:], bg_u[:, t * 128:(t + 1) * 128],
                                        id32[:GU, :GU])
                    nc.scalar.copy(bvnu[:, t * GU:(t + 1) * GU], pt[:])
                for t in range(RT):
                    pt = zps.tile([128, GV], F32, tag="zp", name="zp")
                    nc.tensor.transpose(pt[:], bg_v[:, t * 128:(t + 1) * 128],
                                        id32[:GV, :GV])
                    # bvnv needs the extra *S[r] factor (per-partition)
                    nc.scalar.mul(bvnv[:, t * GV:(t + 1) * GV], pt[:], s_sb[:, t:t + 1])
            for t in range(RT):
                nc.scalar.mul(av[:, t * GV:(t + 1) * GV],
                              svt_sb[:, t * GV:(t + 1) * GV], s_sb[:, t:t + 1])

            # ---- U side ----
            with tc.tile_pool(name="ups", bufs=2, space="PSUM") as ups:
                for t in range(OT):
                    nibu = nibp.tile([128, RANK], I32, tag="nibu", name="nibu")
                    nibu_r = nibu[:].rearrange("p (w j) -> p w j", j=PACK)
                    words = qut_sb[:, t * (RANK // PACK):(t + 1) * (RANK // PACK)]
                    for j in range(PACK):
                        nc.vector.tensor_scalar(
                            out=nibu_r[:, :, j], in0=words, scalar1=4 * j, scalar2=15,
                            op0=Alu.logical_shift_right, op1=Alu.bitwise_and)
                    for g in range(GU):
                        col = t * GU + g
                        affine("u",
                               wut[:, t * RANK + g * 128: t * RANK + (g + 1) * 128],
                               nibu[:, g * 128:(g + 1) * 128],
                               sut_sb[:, col:col + 1], bvnu[:, col:col + 1])
                    for kq in range(2):
                        pt = ups.tile([128, 4096], F16, tag="up", name="up")
                        for kk in range(4):
                            k = kq * 4 + kk
                            nc.tensor.transpose(
                                pt[:, kk * 1024: kk * 1024 + 128],
                                wut[:, t * RANK + k * 128: t * RANK + (k + 1) * 128],
                                id16[:])
                        src = pt.rearrange("p (x c) -> p x c", x=4)[:, :, :128]
                        dsl = lhsT[:].rearrange("p (k o) -> p k o", k=KT)[
                            :, kq * 4:(kq + 1) * 4, t * 128:(t + 1) * 128]
                        copy_alt(dsl, src)

            # ---- V dequant + matmuls ----
            def deq_rt(st, rt):
                rs = rhs[st]
                wps = STRIP // PACK
                nibv = nibp.tile([128, STRIP], I32, tag="nibv", name="nibv")
                nibv_r = nibv[:].rearrange("p (w j) -> p w j", j=PACK)
                words = qvt_sb[:, rt * (I_SL // PACK) + st * wps:
                               rt * (I_SL // PACK) + (st + 1) * wps]
                for j in range(PACK):
                    nc.vector.tensor_scalar(
                        out=nibv_r[:, :, j], in0=words, scalar1=4 * j, scalar2=15,
                        op0=Alu.logical_shift_right, op1=Alu.bitwise_and)
                for gs in range(STRIP // GROUPSIZE):
                    col = rt * GV + st * (STRIP // GROUPSIZE) + gs
                    affine("v",
                           rs[:, rt * STRIP + gs * 128: rt * STRIP + (gs + 1) * 128],
                           nibv[:, gs * 128:(gs + 1) * 128],
                           av[:, col:col + 1], bvnv[:, col:col + 1])

            def mm_k(mps_tiles, st, h, m, k):
                nc.tensor.matmul(
                    mps_tiles[m][:],
                    lhsT[:, k * O_SL + m * 128: k * O_SL + (m + 1) * 128],
                    rhs[st][:, k * STRIP + h * 512: k * STRIP + (h + 1) * 512],
                    start=(k == 0), stop=(k == KT - 1))

            def flush(mps_tiles, st, h, m):
                ot = outp.tile([128, 512], F32, tag="ot", name="ot")
                copy_alt(ot[:], mps_tiles[m][:])
                nc.sync.dma_start(
                    out=out[m * 128:(m + 1) * 128,
                            st * STRIP + h * 512: st * STRIP + (h + 1) * 512],
                    in_=ot[:])

            with tc.tile_pool(name="mps", bufs=8, space="PSUM") as mps:
                for rt in range(RT):
                    deq_rt(0, rt)
                # wave (0,0): k-layered so it can chase strip-0 dequant
                w00 = [mps.tile([128, 512], F32, tag="mm", name="mmps")
                       for _ in range(OT)]
                for k in range(KT):
                    for m in range(OT):
                        mm_k(w00, 0, 0, m, k)
                # strip-1 dequant; wave-(0,0) flushes right after (they hit the
                # engine FIFOs once dequant is queued, and their MMs are done)
                for x in range(RT):
                    deq_rt(1, x)
                    if x % 2 == 1:
                        flush(w00, 0, 0, x - 1)
                        flush(w00, 0, 0, x)
                # wave (0,1) m-grouped with inline flush (psum freed above)
                t01 = {}
                for m in range(OT):
                    t01[m] = mps.tile([128, 512], F32, tag="mm", name="mmps")
                    for k in range(KT):
                        mm_k(t01, 0, 1, m, k)
                    flush(t01, 0, 1, m)
                # wave (1,0): k-layered to chase strip-1 dequant tail
                w10 = [mps.tile([128, 512], F32, tag="mm", name="mmps")
                       for _ in range(OT)]
                for k in range(KT):
                    for m in range(OT):
                        mm_k(w10, 1, 0, m, k)
                for m in range(OT):
                    flush(w10, 1, 0, m)
                t11 = {}
                for m in range(OT):
                    t11[m] = mps.tile([128, 512], F32, tag="mm", name="mmps")
                    for k in range(KT):
                        mm_k(t11, 1, 1, m, k)
                    flush(t11, 1, 1, m)

    nc.compile()
    return nc


def _host_prep(qweight_V, qzeros_V, scales_V, qweight_U, qzeros_U, scales_U, S):
    """Layout-only host prep: slice/transpose packed words + tables into SBUF layouts."""
    in_maps = []
    for c in range(N_CORES):
        a, b = divmod(c, P_I)
        qv = qweight_V[b * (I_SL // PACK):(b + 1) * (I_SL // PACK), :]
        qvt_h = np.ascontiguousarray(
            qv.T.reshape(RT, 128, I_SL // PACK).transpose(1, 0, 2).reshape(128, -1))
        sv = scales_V.T[:, b * GV:(b + 1) * GV]
        svt_h = np.ascontiguousarray(
            sv.reshape(RT, 128, GV).transpose(1, 0, 2).reshape(128, -1))
        qzv_h = np.ascontiguousarray(qzeros_V[b * GV:(b + 1) * GV, :])
        sv_g_h = np.ascontiguousarray(scales_V[b * GV:(b + 1) * GV, :])
        qu = qweight_U[:, a * O_SL:(a + 1) * O_SL]
        qut_h = np.ascontiguousarray(
            qu.T.reshape(OT, 128, RANK // PACK).transpose(1, 0, 2).reshape(128, -1))
        su = scales_U.T[a * O_SL:(a + 1) * O_SL, :]
        sut_h = np.ascontiguousarray(
            su.reshape(OT, 128, GU).transpose(1, 0, 2).reshape(128, -1))
        qzu_h = np.ascontiguousarray(qzeros_U[:, a * (O_SL // PACK):(a + 1) * (O_SL // PACK)])
        su_g_h = np.ascontiguousarray(scales_U[:, a * O_SL:(a + 1) * O_SL])
        s_h = np.ascontiguousarray(S.reshape(RT, 128).T)
        in_maps.append({
            "qvt": qvt_h, "svt": svt_h, "qzv": qzv_h, "sv_g": sv_g_h,
            "qut": qut_h, "sut": sut_h, "qzu": qzu_h, "su_g": su_g_h, "s": s_h,
        })
    return in_maps


def kernel(x, qweight_V, qzeros_V, scales_V, g_idx_V,
           qweight_U, qzeros_U, scales_U, g_idx_U, S, **_unused):
    global _NC_CACHE, LAST_RESULTS
    qweight_V = np.asarray(qweight_V, dtype=np.int32)
    qzeros_V = np.asarray(qzeros_V, dtype=np.int32)
    scales_V = np.asarray(scales_V, dtype=np.float32)
    qweight_U = np.asarray(qweight_U, dtype=np.int32)
    qzeros_U = np.asarray(qzeros_U, dtype=np.int32)
    scales_U = np.asarray(scales_U, dtype=np.float32)
    S = np.asarray(S, dtype=np.float32)

    if _NC_CACHE is None:
        _NC_CACHE = _build_nc()
    nc = _NC_CACHE

    in_maps = _host_prep(qweight_V, qzeros_V, scales_V,
                         qweight_U, qzeros_U, scales_U, S)
    res = run_bass_kernel_spmd(nc, in_maps, core_ids=list(range(N_CORES)), trace=TRACE)
    LAST_RESULTS = res

    O = np.empty((OUT_SIZE, IN_SIZE), dtype=np.float32)
    for c in range(N_CORES):
        a, b = divmod(c, P_I)
        O[a * O_SL:(a + 1) * O_SL, b * I_SL:(b + 1) * I_SL] = res.results[c]["out"]
    return O

